# revision 11
# baseline (speedup 1.0000x reference)
"""Expert-parallel MoE kernel for Trainium2 (8 NeuronCores).

Strategy (matches the expert-parallel sharding hint):
  - Router is evaluated on host with the exact same jax ops as the
    reference (same backend) so top-k decisions match bit-for-bit.
  - Tokens are dispatched (gathered) per expert on host; each of the 8
    cores owns one expert's weights and runs a fused MLP
        Y = (silu(X @ G^T) * (X @ U^T)) @ D^T
    over its gathered tokens in bf16 (fp32 PSUM accumulate).
  - Outputs are combined on host: out[token] += mean_w[e] * Y_e[row].

v2 schedule: tokens are processed in GROUPS of ~1024-1152 (sub-tiles of
<=512 for PSUM) so each gate/up weight block streams from HBM only once
per group (2 passes total at C=2176 instead of 5).  This keeps the
required stage-1 weight bandwidth at ~76 GB/s (vs 153+ before, and 410
for the old 128-token tail, which was DMA-bound).  The down-projection
weights stay SBUF-resident; their (re)load is interleaved into the
middle of the first stage-1 pass so they never delay the critical path.
"""

import sys
from contextlib import ExitStack

if "/opt/trn_rl_repo" not in sys.path:
    sys.path.insert(0, "/opt/trn_rl_repo")

import ml_dtypes
import numpy as np

import concourse.bacc as bacc
import concourse.mybir as mybir
import concourse.tile as tile
from concourse.bass_utils import run_bass_kernel_spmd

B, S, H, I, E, TOPK = 4, 2048, 1024, 4096, 8, 2
T = B * S
KCH = H // 128   # 8 contraction chunks over H
IB = I // 128    # 32 blocks over I
BF16 = mybir.dt.bfloat16
F32 = mybir.dt.float32
ACT_FN = mybir.ActivationFunctionType.Silu

_prog_cache: dict[tuple, object] = {}


def _groups(C):
    """Split capacity C (multiple of 128) into groups of <=1152 tokens,
    each a list of (start, size<=512) sub-tiles."""
    gs = []
    c = 0
    while C - c > 1152:
        gs.append((c, 1024))
        c += 1024
    gs.append((c, C - c))
    out = []
    for g0, gl in gs:
        # near-equal sub-tiles (multiples of 128, <=512) so the matmul free
        # dim stays >=256 and LDWEIGHTS is always hidden (1152 -> 3x384)
        n = -(-gl // 512)
        base = gl // n // 128 * 128
        sizes = [base] * n
        for i in range((gl - base * n) // 128):
            sizes[i] += 128
        subs = []
        s = 0
        for L in sizes:
            subs.append((g0 + s, L))
            s += L
        out.append((g0, gl, subs))
    return out


def build_program(C, reps=1):
    key = (C, reps)
    if key in _prog_cache:
        return _prog_cache[key]
    nc = bacc.Bacc("TRN2", target_bir_lowering=False, debug=False, num_devices=8)

    xt_d = nc.dram_tensor("xt", [128, KCH, C], BF16, kind="ExternalInput").ap()
    gut_d = nc.dram_tensor("gut", [IB, 128, 2, KCH, 128], BF16, kind="ExternalInput").ap()
    dt_d = nc.dram_tensor("dt", [4, 128, IB // 4, H], BF16, kind="ExternalInput").ap()
    y_d = nc.dram_tensor("y", [C, H], F32, kind="ExternalOutput").ap()

    with tile.TileContext(nc) as tc:
        with ExitStack() as stack:
            if reps > 1:
                # hint_engines: body is ~29 IRAM blocks on PE; the hint makes
                # the back-edge branch I$-hit (~300ns) instead of ~4us.
                stack.enter_context(
                    tc.For_i(0, reps, 1, hint_engines=(mybir.EngineType.PE,))
                )
            _emit_body(nc, tc, C, xt_d, gut_d, dt_d, y_d)

    nc.compile()
    _prog_cache[key] = nc
    return nc


def _emit_body(nc, tc, C, xt_d, gut_d, dt_d, y_d):
    groups = _groups(C)
    with (
        tc.tile_pool(name="wpool", bufs=4) as wpool,
        tc.tile_pool(name="xpool", bufs=1) as xpool,
        tc.tile_pool(name="dpool", bufs=1) as dpool,
        tc.tile_pool(name="hpool", bufs=1) as hpool,
        tc.tile_pool(name="spool", bufs=3) as spool,
        tc.tile_pool(name="ypool", bufs=3) as ypool,
        tc.tile_pool(name="psum", bufs=2, space="PSUM") as psum,
    ):
        # D^T resident: 4 chunks x [128, 8, 1024] bf16 (2MB DMAs), loaded
        # lazily: chunk g's DMA is issued mid-way through the first
        # stage-1 pass so it never queues ahead of xt/gut on the wires.
        dts = [
            dpool.tile([128, IB // 4, H], BF16, tag=f"dt{g}", name=f"dt{g}")
            for g in range(4)
        ]
        dt_loaded = [False] * 4

        first = True
        for g0, gl, subs in groups:
            # token activations for this group: one DMA piece per sub-tile so
            # the first sub's matmuls can start before the whole group lands
            # (the first gut block is issued ahead of the bulk of xt).
            xt = xpool.tile([128, KCH, gl], BF16, tag="xt")
            guts = [wpool.tile([128, 2, KCH, 128], BF16, tag="gut", name=f"gut{ib}")
                    for ib in range(2)]
            nc.sync.dma_start(guts[0][:], gut_d[0])
            for s0, sl_len in subs:
                ls = s0 - g0
                nc.sync.dma_start(
                    xt[:, :, ls:ls + sl_len], xt_d[:, :, s0:s0 + sl_len]
                )
            nc.sync.dma_start(guts[1][:], gut_d[1])

            # stage 1: hh[ib][:, c] = silu(G x) * (U x) for the whole group
            hhs = []
            for ib in range(IB):
                if ib < 2:
                    gut = guts[ib]
                else:
                    gut = wpool.tile([128, 2, KCH, 128], BF16, tag="gut")
                    nc.sync.dma_start(gut[:], gut_d[ib])
                if first and ib % 8 == 4:
                    g = ib // 8
                    nc.sync.dma_start(dts[g][:], dt_d[g])
                    dt_loaded[g] = True
                hh = hpool.tile([128, gl], BF16, tag=f"hh{ib}")
                for s0, sl_len in subs:
                    ls = s0 - g0
                    a1 = psum.tile([128, sl_len], F32, tag="a1")
                    for k in range(KCH):
                        nc.tensor.matmul(
                            a1[:], gut[:, 0, k, :], xt[:, k, ls:ls + sl_len],
                            start=(k == 0), stop=(k == KCH - 1),
                        )
                    a2 = psum.tile([128, sl_len], F32, tag="a2")
                    for k in range(KCH):
                        nc.tensor.matmul(
                            a2[:], gut[:, 1, k, :], xt[:, k, ls:ls + sl_len],
                            start=(k == 0), stop=(k == KCH - 1),
                        )
                    sl = spool.tile([128, sl_len], F32, tag="silu")
                    nc.scalar.activation(sl[:], a1[:], ACT_FN)
                    nc.vector.tensor_mul(hh[:, ls:ls + sl_len], sl[:], a2[:])
                hhs.append(hh)
            if first:
                for g in range(4):
                    if not dt_loaded[g]:
                        nc.sync.dma_start(dts[g][:], dt_d[g])
                        dt_loaded[g] = True
                first = False

            # stage 2: Y[c, h] = Hh @ D^T  (contract I), write [w,1024] rows
            for cs0 in range(0, gl, 128):
                w = min(128, gl - cs0)
                yt = ypool.tile([w, H], F32, tag="yt")
                for h0 in range(0, H, 512):
                    py = psum.tile([w, 512], F32, tag="py")
                    for ic in range(IB):
                        nc.tensor.matmul(
                            py[:],
                            hhs[ic][:, cs0:cs0 + w],
                            dts[ic // 8][:, ic % 8, h0:h0 + 512],
                            start=(ic == 0), stop=(ic == IB - 1),
                        )
                    nc.scalar.copy(yt[:, h0:h0 + 512], py[:])
                nc.sync.dma_start(y_d[g0 + cs0:g0 + cs0 + w, :], yt[:])


def _routing(x, router_w):
    """Replicate the reference's routing decisions with identical jax ops."""
    import jax
    import jax.numpy as jnp

    xf = jnp.asarray(x).reshape(-1, H)
    logits = xf @ jnp.asarray(router_w).T
    probs = jax.nn.softmax(logits, axis=-1)
    topk_p, topk_i = jax.lax.top_k(probs, TOPK)
    topk_p = topk_p / topk_p.sum(axis=-1, keepdims=True)
    return np.asarray(topk_p), np.asarray(topk_i)


def prepare(x, router_w, gate_w, up_w, down_w):
    """Host-side dispatch: returns (nc, in_maps, combine) where combine maps
    the per-core device outputs to the full [B,S,H] result."""
    topk_p, topk_i = _routing(x, router_w)
    xf = np.ascontiguousarray(np.asarray(x, dtype=np.float32).reshape(T, H))

    idxs, weights = [], []
    for e in range(E):
        sel = topk_i == e
        mask = sel.any(axis=-1)
        w_tok = (topk_p * sel).sum(axis=-1)
        cnt = int(mask.sum())
        mean_w = float(w_tok.sum() / max(cnt, 1)) if cnt > 0 else 0.0
        idxs.append(np.nonzero(mask)[0])
        weights.append(np.float32(mean_w))

    cmax = max(len(ix) for ix in idxs)
    C = ((cmax + 127) // 128) * 128

    xf_bf = xf.astype(ml_dtypes.bfloat16)
    in_maps = []
    for e in range(E):
        ix = idxs[e]
        # X^T packed [128(p), KCH(k), C(c)] with h = k*128+p
        xt = np.zeros((128, KCH, C), dtype=ml_dtypes.bfloat16)
        xt[:, :, : len(ix)] = xf_bf[ix].T.reshape(KCH, 128, len(ix)).transpose(1, 0, 2)
        # G^T/U^T are [H, I]; packed together as
        # [IB, 128(p), 2(g/u), KCH(k), 128(i)] with h = k*128+p, so each
        # ib block is one contiguous 512KB DMA with 4KB per partition.
        gT = np.asarray(gate_w[e], dtype=np.float32).T.astype(ml_dtypes.bfloat16)
        uT = np.asarray(up_w[e], dtype=np.float32).T.astype(ml_dtypes.bfloat16)
        gt = gT.reshape(KCH, 128, IB, 128).transpose(2, 1, 0, 3)
        ut = uT.reshape(KCH, 128, IB, 128).transpose(2, 1, 0, 3)
        gut = np.ascontiguousarray(np.stack([gt, ut], axis=2))
        # D^T is [I, H]; packed [4, 128(p over I), 8(ib in chunk), H] with
        # i = (g*8 + ib)*128 + p, so each chunk is one contiguous 2MB DMA
        dT = np.asarray(down_w[e], dtype=np.float32).T.astype(ml_dtypes.bfloat16)
        dt = np.ascontiguousarray(
            dT.reshape(4, IB // 4, 128, H).transpose(0, 2, 1, 3)
        )
        in_maps.append({"xt": xt, "gut": gut, "dt": dt})

    nc = build_program(C)

    def combine(results):
        out = np.zeros((T, H), dtype=np.float32)
        for e in range(E):
            ix = idxs[e]
            y = results[e]["y"]
            out[ix] += weights[e] * y[: len(ix)]
        return out.reshape(B, S, H)

    return nc, in_maps, combine


def kernel(x, router_w, gate_w, up_w, down_w):
    nc, in_maps, combine = prepare(x, router_w, gate_w, up_w, down_w)
    res = run_bass_kernel_spmd(nc, in_maps, list(range(8)))
    return combine(res.results)


# revision 13
# speedup vs baseline: 1.0082x; 1.0082x over previous
"""Expert-parallel MoE kernel for Trainium2 (8 NeuronCores).

Strategy (matches the expert-parallel sharding hint):
  - Router is evaluated on host with the exact same jax ops as the
    reference (same backend) so top-k decisions match bit-for-bit.
  - Tokens are dispatched (gathered) per expert on host; each of the 8
    cores owns one expert's weights and runs a fused MLP
        Y = (silu(X @ G^T) * (X @ U^T)) @ D^T
    over its gathered tokens in bf16 (fp32 PSUM accumulate).
  - Outputs are combined on host: out[token] += mean_w[e] * Y_e[row].

v2 schedule: tokens are processed in GROUPS of ~1024-1152 (sub-tiles of
<=512 for PSUM) so each gate/up weight block streams from HBM only once
per group (2 passes total at C=2176 instead of 5).  This keeps the
required stage-1 weight bandwidth at ~76 GB/s (vs 153+ before, and 410
for the old 128-token tail, which was DMA-bound).  The down-projection
weights stay SBUF-resident; their (re)load is interleaved into the
middle of the first stage-1 pass so they never delay the critical path.
"""

import sys
from contextlib import ExitStack

if "/opt/trn_rl_repo" not in sys.path:
    sys.path.insert(0, "/opt/trn_rl_repo")

import ml_dtypes
import numpy as np

import concourse.bacc as bacc
import concourse.mybir as mybir
import concourse.tile as tile
from concourse.bass_utils import run_bass_kernel_spmd

B, S, H, I, E, TOPK = 4, 2048, 1024, 4096, 8, 2
T = B * S
KCH = H // 128   # 8 contraction chunks over H
IB = I // 128    # 32 blocks over I
BF16 = mybir.dt.bfloat16
F32 = mybir.dt.float32
ACT_FN = mybir.ActivationFunctionType.Silu

_prog_cache: dict[tuple, object] = {}


def _groups(C):
    """Split capacity C (multiple of 128) into groups of <=1152 tokens,
    each a list of (start, size<=512) sub-tiles."""
    gs = []
    c = 0
    while C - c > 1152:
        gs.append((c, 1024))
        c += 1024
    gs.append((c, C - c))
    out = []
    for g0, gl in gs:
        # near-equal sub-tiles (multiples of 128, <=512) so the matmul free
        # dim stays >=256 and LDWEIGHTS is always hidden (1152 -> 3x384)
        n = -(-gl // 512)
        base = gl // n // 128 * 128
        sizes = [base] * n
        for i in range((gl - base * n) // 128):
            sizes[i] += 128
        subs = []
        s = 0
        for L in sizes:
            subs.append((g0 + s, L))
            s += L
        out.append((g0, gl, subs))
    return out


def build_program(C, reps=1):
    key = (C, reps)
    if key in _prog_cache:
        return _prog_cache[key]
    nc = bacc.Bacc("TRN2", target_bir_lowering=False, debug=False, num_devices=8)

    xt_d = nc.dram_tensor("xt", [128, KCH, C], BF16, kind="ExternalInput").ap()
    gut_d = nc.dram_tensor("gut", [IB, 128, 2, KCH, 128], BF16, kind="ExternalInput").ap()
    dt_d = nc.dram_tensor("dt", [4, 128, IB // 4, H], BF16, kind="ExternalInput").ap()
    y_d = nc.dram_tensor("y", [C, H], F32, kind="ExternalOutput").ap()

    with tile.TileContext(nc) as tc:
        with ExitStack() as stack:
            if reps > 1:
                # hint_engines: body is ~29 IRAM blocks on PE; the hint makes
                # the back-edge branch I$-hit (~300ns) instead of ~4us.
                stack.enter_context(
                    tc.For_i(0, reps, 1, hint_engines=(mybir.EngineType.PE,))
                )
            _emit_body(nc, tc, C, xt_d, gut_d, dt_d, y_d)

    nc.compile()
    _prog_cache[key] = nc
    return nc


def _emit_body(nc, tc, C, xt_d, gut_d, dt_d, y_d):
    groups = _groups(C)
    with (
        tc.tile_pool(name="wpool", bufs=6) as wpool,
        tc.tile_pool(name="xpool", bufs=1) as xpool,
        tc.tile_pool(name="dpool", bufs=1) as dpool,
        tc.tile_pool(name="hpool", bufs=1) as hpool,
        tc.tile_pool(name="spool", bufs=3) as spool,
        tc.tile_pool(name="ypool", bufs=2) as ypool,
        tc.tile_pool(name="psum", bufs=2, space="PSUM") as psum,
    ):
        # D^T resident: 4 chunks x [128, 8, 1024] bf16 (2MB DMAs), loaded
        # lazily: chunk g's DMA is issued mid-way through the first
        # stage-1 pass so it never queues ahead of xt/gut on the wires.
        dts = [
            dpool.tile([128, IB // 4, H], BF16, tag=f"dt{g}", name=f"dt{g}")
            for g in range(4)
        ]
        dt_loaded = [False] * 4

        first = True
        for g0, gl, subs in groups:
            # token activations for this group: one DMA piece per sub-tile so
            # the first sub's matmuls can start before the whole group lands
            # (the first gut block is issued ahead of the bulk of xt).
            xt = xpool.tile([128, KCH, gl], BF16, tag="xt")
            guts = [wpool.tile([128, 2, KCH, 128], BF16, tag="gut", name=f"gut{ib}")
                    for ib in range(2)]
            nc.sync.dma_start(guts[0][:], gut_d[0])
            for s0, sl_len in subs:
                ls = s0 - g0
                nc.sync.dma_start(
                    xt[:, :, ls:ls + sl_len], xt_d[:, :, s0:s0 + sl_len]
                )
            nc.sync.dma_start(guts[1][:], gut_d[1])

            # stage 1: hh[ib][:, c] = silu(G x) * (U x) for the whole group
            hhs = []
            for ib in range(IB):
                if ib < 2:
                    gut = guts[ib]
                else:
                    gut = wpool.tile([128, 2, KCH, 128], BF16, tag="gut")
                    nc.sync.dma_start(gut[:], gut_d[ib])
                if first and ib % 8 == 4:
                    g = ib // 8
                    nc.sync.dma_start(dts[g][:], dt_d[g])
                    dt_loaded[g] = True
                hh = hpool.tile([128, gl], BF16, tag=f"hh{ib}")
                for s0, sl_len in subs:
                    ls = s0 - g0
                    a1 = psum.tile([128, sl_len], F32, tag="a1")
                    for k in range(KCH):
                        nc.tensor.matmul(
                            a1[:], gut[:, 0, k, :], xt[:, k, ls:ls + sl_len],
                            start=(k == 0), stop=(k == KCH - 1),
                        )
                    a2 = psum.tile([128, sl_len], F32, tag="a2")
                    for k in range(KCH):
                        nc.tensor.matmul(
                            a2[:], gut[:, 1, k, :], xt[:, k, ls:ls + sl_len],
                            start=(k == 0), stop=(k == KCH - 1),
                        )
                    sl = spool.tile([128, sl_len], F32, tag="silu")
                    nc.scalar.activation(sl[:], a1[:], ACT_FN)
                    nc.vector.tensor_mul(hh[:, ls:ls + sl_len], sl[:], a2[:])
                hhs.append(hh)
            if first:
                for g in range(4):
                    if not dt_loaded[g]:
                        nc.sync.dma_start(dts[g][:], dt_d[g])
                        dt_loaded[g] = True
                first = False

            # stage 2: Y[c, h] = Hh @ D^T  (contract I), write [w,1024] rows
            for cs0 in range(0, gl, 128):
                w = min(128, gl - cs0)
                yt = ypool.tile([w, H], F32, tag="yt")
                for h0 in range(0, H, 512):
                    py = psum.tile([w, 512], F32, tag="py", bufs=3)
                    for ic in range(IB):
                        nc.tensor.matmul(
                            py[:],
                            hhs[ic][:, cs0:cs0 + w],
                            dts[ic // 8][:, ic % 8, h0:h0 + 512],
                            start=(ic == 0), stop=(ic == IB - 1),
                        )
                    nc.scalar.copy(yt[:, h0:h0 + 512], py[:])
                nc.sync.dma_start(y_d[g0 + cs0:g0 + cs0 + w, :], yt[:])


def _routing(x, router_w):
    """Replicate the reference's routing decisions with identical jax ops."""
    import jax
    import jax.numpy as jnp

    xf = jnp.asarray(x).reshape(-1, H)
    logits = xf @ jnp.asarray(router_w).T
    probs = jax.nn.softmax(logits, axis=-1)
    topk_p, topk_i = jax.lax.top_k(probs, TOPK)
    topk_p = topk_p / topk_p.sum(axis=-1, keepdims=True)
    return np.asarray(topk_p), np.asarray(topk_i)


def prepare(x, router_w, gate_w, up_w, down_w):
    """Host-side dispatch: returns (nc, in_maps, combine) where combine maps
    the per-core device outputs to the full [B,S,H] result."""
    topk_p, topk_i = _routing(x, router_w)
    xf = np.ascontiguousarray(np.asarray(x, dtype=np.float32).reshape(T, H))

    idxs, weights = [], []
    for e in range(E):
        sel = topk_i == e
        mask = sel.any(axis=-1)
        w_tok = (topk_p * sel).sum(axis=-1)
        cnt = int(mask.sum())
        mean_w = float(w_tok.sum() / max(cnt, 1)) if cnt > 0 else 0.0
        idxs.append(np.nonzero(mask)[0])
        weights.append(np.float32(mean_w))

    cmax = max(len(ix) for ix in idxs)
    C = ((cmax + 127) // 128) * 128

    xf_bf = xf.astype(ml_dtypes.bfloat16)
    in_maps = []
    for e in range(E):
        ix = idxs[e]
        # X^T packed [128(p), KCH(k), C(c)] with h = k*128+p
        xt = np.zeros((128, KCH, C), dtype=ml_dtypes.bfloat16)
        xt[:, :, : len(ix)] = xf_bf[ix].T.reshape(KCH, 128, len(ix)).transpose(1, 0, 2)
        # G^T/U^T are [H, I]; packed together as
        # [IB, 128(p), 2(g/u), KCH(k), 128(i)] with h = k*128+p, so each
        # ib block is one contiguous 512KB DMA with 4KB per partition.
        gT = np.asarray(gate_w[e], dtype=np.float32).T.astype(ml_dtypes.bfloat16)
        uT = np.asarray(up_w[e], dtype=np.float32).T.astype(ml_dtypes.bfloat16)
        gt = gT.reshape(KCH, 128, IB, 128).transpose(2, 1, 0, 3)
        ut = uT.reshape(KCH, 128, IB, 128).transpose(2, 1, 0, 3)
        gut = np.ascontiguousarray(np.stack([gt, ut], axis=2))
        # D^T is [I, H]; packed [4, 128(p over I), 8(ib in chunk), H] with
        # i = (g*8 + ib)*128 + p, so each chunk is one contiguous 2MB DMA
        dT = np.asarray(down_w[e], dtype=np.float32).T.astype(ml_dtypes.bfloat16)
        dt = np.ascontiguousarray(
            dT.reshape(4, IB // 4, 128, H).transpose(0, 2, 1, 3)
        )
        in_maps.append({"xt": xt, "gut": gut, "dt": dt})

    nc = build_program(C)

    def combine(results):
        out = np.zeros((T, H), dtype=np.float32)
        for e in range(E):
            ix = idxs[e]
            y = results[e]["y"]
            out[ix] += weights[e] * y[: len(ix)]
        return out.reshape(B, S, H)

    return nc, in_maps, combine


def kernel(x, router_w, gate_w, up_w, down_w):
    nc, in_maps, combine = prepare(x, router_w, gate_w, up_w, down_w)
    res = run_bass_kernel_spmd(nc, in_maps, list(range(8)))
    return combine(res.results)


# revision 18
# speedup vs baseline: 1.0503x; 1.0418x over previous
"""Expert-parallel MoE kernel for Trainium2 (8 NeuronCores).

Strategy (matches the expert-parallel sharding hint):
  - Router is evaluated on host with the exact same jax ops as the
    reference (same backend) so top-k decisions match bit-for-bit.
  - Tokens are dispatched (gathered) per expert on host; each of the 8
    cores owns one expert's weights and runs a fused MLP
        Y = (silu(X @ G^T) * (X @ U^T)) @ D^T
    over its gathered tokens in bf16 (fp32 PSUM accumulate).
  - Outputs are combined on host: out[token] += mean_w[e] * Y_e[row].

v2 schedule: tokens are processed in GROUPS of ~1024-1152 (sub-tiles of
<=512 for PSUM) so each gate/up weight block streams from HBM only once
per group (2 passes total at C=2176 instead of 5).  This keeps the
required stage-1 weight bandwidth at ~76 GB/s (vs 153+ before, and 410
for the old 128-token tail, which was DMA-bound).  The down-projection
weights stay SBUF-resident; their (re)load is interleaved into the
middle of the first stage-1 pass so they never delay the critical path.
"""

import sys
from contextlib import ExitStack

if "/opt/trn_rl_repo" not in sys.path:
    sys.path.insert(0, "/opt/trn_rl_repo")

import ml_dtypes
import numpy as np

import concourse.bacc as bacc
import concourse.mybir as mybir
import concourse.tile as tile
from concourse.bass_utils import run_bass_kernel_spmd

B, S, H, I, E, TOPK = 4, 2048, 1024, 4096, 8, 2
T = B * S
KCH = H // 128   # 8 contraction chunks over H
IB = I // 128    # 32 blocks over I
BF16 = mybir.dt.bfloat16
F32 = mybir.dt.float32
ACT_FN = mybir.ActivationFunctionType.Silu

_prog_cache: dict[tuple, object] = {}


def _groups(C):
    """Split capacity C (multiple of 128) into groups of <=1152 tokens,
    each a list of (start, size<=512) sub-tiles."""
    gs = []
    c = 0
    while C - c > 1152:
        gs.append((c, 1024))
        c += 1024
    gs.append((c, C - c))
    out = []
    for g0, gl in gs:
        # near-equal sub-tiles (multiples of 128, <=512) so the matmul free
        # dim stays >=256 and LDWEIGHTS is always hidden (1152 -> 3x384)
        n = -(-gl // 512)
        base = gl // n // 128 * 128
        sizes = [base] * n
        for i in range((gl - base * n) // 128):
            sizes[i] += 128
        subs = []
        s = 0
        for L in sizes:
            subs.append((g0 + s, L))
            s += L
        out.append((g0, gl, subs))
    return out


def build_program(C, reps=1):
    key = (C, reps)
    if key in _prog_cache:
        return _prog_cache[key]
    nc = bacc.Bacc("TRN2", target_bir_lowering=False, debug=False, num_devices=8)

    xt_d = nc.dram_tensor("xt", [128, KCH, C], BF16, kind="ExternalInput").ap()
    gut_d = nc.dram_tensor("gut", [IB, 128, 2, KCH, 128], BF16, kind="ExternalInput").ap()
    dt_d = nc.dram_tensor("dt", [4, 128, IB // 4, H], BF16, kind="ExternalInput").ap()
    y_d = nc.dram_tensor("y", [C, H], F32, kind="ExternalOutput").ap()

    with tile.TileContext(nc) as tc:
        with ExitStack() as stack:
            if reps > 1:
                # hint_engines: body is ~29 IRAM blocks on PE; the hint
                # makes the back-edge branch I$-hit instead of ~4us.
                # (2x loop unrolling via For_i_unrolled_general was tried
                # and faulted the exec unit on HW despite passing CoreSim -
                # do not re-attempt without a device-level debug path.)
                stack.enter_context(
                    tc.For_i(0, reps, 1, hint_engines=(mybir.EngineType.PE,))
                )
            _emit_body(nc, tc, C, xt_d, gut_d, dt_d, y_d)

    nc.compile()
    _prog_cache[key] = nc
    return nc


def _emit_body(nc, tc, C, xt_d, gut_d, dt_d, y_d):
    groups = _groups(C)
    with (
        tc.tile_pool(name="wpool", bufs=6) as wpool,
        tc.tile_pool(name="xpool", bufs=1) as xpool,
        tc.tile_pool(name="dpool", bufs=1) as dpool,
        tc.tile_pool(name="hpool", bufs=1) as hpool,
        tc.tile_pool(name="spool", bufs=3) as spool,
        tc.tile_pool(name="ypool", bufs=2) as ypool,
        tc.tile_pool(name="psum", bufs=2, space="PSUM") as psum,
    ):
        # D^T resident: 4 chunks x [128, 8, 1024] bf16 (2MB DMAs), loaded
        # lazily: chunk g's DMA is issued mid-way through the first
        # stage-1 pass so it never queues ahead of xt/gut on the wires.
        dts = [
            dpool.tile([128, IB // 4, H], BF16, tag=f"dt{g}", name=f"dt{g}")
            for g in range(4)
        ]
        dt_loaded = [False] * 4

        first = True
        for g0, gl, subs in groups:
            # token activations for this group: one DMA piece per sub-tile so
            # the first sub's matmuls can start before the whole group lands
            # (the first gut block is issued ahead of the bulk of xt).
            xt = xpool.tile([128, KCH, gl], BF16, tag="xt")
            guts = [wpool.tile([128, 2, KCH, 128], BF16, tag="gut", name=f"gut{ib}")
                    for ib in range(2)]
            nc.sync.dma_start(guts[0][:], gut_d[0])
            for s0, sl_len in subs:
                ls = s0 - g0
                nc.sync.dma_start(
                    xt[:, :, ls:ls + sl_len], xt_d[:, :, s0:s0 + sl_len]
                )
            nc.sync.dma_start(guts[1][:], gut_d[1])

            # stage 1: hh[ib][:, c] = silu(G x) * (U x) for the whole group
            hhs = []
            for ib in range(IB):
                if ib < 2:
                    gut = guts[ib]
                else:
                    gut = wpool.tile([128, 2, KCH, 128], BF16, tag="gut")
                    nc.sync.dma_start(gut[:], gut_d[ib])
                if first and ib % 8 == 4:
                    g = ib // 8
                    nc.sync.dma_start(dts[g][:], dt_d[g])
                    dt_loaded[g] = True
                hh = hpool.tile([128, gl], BF16, tag=f"hh{ib}")
                for s0, sl_len in subs:
                    ls = s0 - g0
                    a1 = psum.tile([128, sl_len], F32, tag="a1")
                    for k in range(KCH):
                        nc.tensor.matmul(
                            a1[:], gut[:, 0, k, :], xt[:, k, ls:ls + sl_len],
                            start=(k == 0), stop=(k == KCH - 1),
                        )
                    a2 = psum.tile([128, sl_len], F32, tag="a2")
                    for k in range(KCH):
                        nc.tensor.matmul(
                            a2[:], gut[:, 1, k, :], xt[:, k, ls:ls + sl_len],
                            start=(k == 0), stop=(k == KCH - 1),
                        )
                    sl = spool.tile([128, sl_len], F32, tag="silu")
                    nc.scalar.activation(sl[:], a1[:], ACT_FN)
                    nc.vector.tensor_mul(hh[:, ls:ls + sl_len], sl[:], a2[:])
                hhs.append(hh)
            if first:
                for g in range(4):
                    if not dt_loaded[g]:
                        nc.sync.dma_start(dts[g][:], dt_d[g])
                        dt_loaded[g] = True
                first = False

            # stage 2: Y[c, h] = Hh @ D^T  (contract I), write [w,1024] rows
            for cs0 in range(0, gl, 128):
                w = min(128, gl - cs0)
                yt = ypool.tile([w, H], F32, tag="yt")
                for h0 in range(0, H, 512):
                    py = psum.tile([w, 512], F32, tag="py", bufs=3)
                    for ic in range(IB):
                        nc.tensor.matmul(
                            py[:],
                            hhs[ic][:, cs0:cs0 + w],
                            dts[ic // 8][:, ic % 8, h0:h0 + 512],
                            start=(ic == 0), stop=(ic == IB - 1),
                        )
                    nc.scalar.copy(yt[:, h0:h0 + 512], py[:])
                nc.sync.dma_start(y_d[g0 + cs0:g0 + cs0 + w, :], yt[:])


def _routing(x, router_w):
    """Replicate the reference's routing decisions with identical jax ops."""
    import jax
    import jax.numpy as jnp

    xf = jnp.asarray(x).reshape(-1, H)
    logits = xf @ jnp.asarray(router_w).T
    probs = jax.nn.softmax(logits, axis=-1)
    topk_p, topk_i = jax.lax.top_k(probs, TOPK)
    topk_p = topk_p / topk_p.sum(axis=-1, keepdims=True)
    return np.asarray(topk_p), np.asarray(topk_i)


def prepare(x, router_w, gate_w, up_w, down_w):
    """Host-side dispatch: returns (nc, in_maps, combine) where combine maps
    the per-core device outputs to the full [B,S,H] result."""
    topk_p, topk_i = _routing(x, router_w)
    xf = np.ascontiguousarray(np.asarray(x, dtype=np.float32).reshape(T, H))

    idxs, weights = [], []
    for e in range(E):
        sel = topk_i == e
        mask = sel.any(axis=-1)
        w_tok = (topk_p * sel).sum(axis=-1)
        cnt = int(mask.sum())
        mean_w = float(w_tok.sum() / max(cnt, 1)) if cnt > 0 else 0.0
        idxs.append(np.nonzero(mask)[0])
        weights.append(np.float32(mean_w))

    cmax = max(len(ix) for ix in idxs)
    C = ((cmax + 127) // 128) * 128

    xf_bf = xf.astype(ml_dtypes.bfloat16)
    in_maps = []
    for e in range(E):
        ix = idxs[e]
        # X^T packed [128(p), KCH(k), C(c)] with h = k*128+p
        xt = np.zeros((128, KCH, C), dtype=ml_dtypes.bfloat16)
        xt[:, :, : len(ix)] = xf_bf[ix].T.reshape(KCH, 128, len(ix)).transpose(1, 0, 2)
        # G^T/U^T are [H, I]; packed together as
        # [IB, 128(p), 2(g/u), KCH(k), 128(i)] with h = k*128+p, so each
        # ib block is one contiguous 512KB DMA with 4KB per partition.
        gT = np.asarray(gate_w[e], dtype=np.float32).T.astype(ml_dtypes.bfloat16)
        uT = np.asarray(up_w[e], dtype=np.float32).T.astype(ml_dtypes.bfloat16)
        gt = gT.reshape(KCH, 128, IB, 128).transpose(2, 1, 0, 3)
        ut = uT.reshape(KCH, 128, IB, 128).transpose(2, 1, 0, 3)
        gut = np.ascontiguousarray(np.stack([gt, ut], axis=2))
        # D^T is [I, H]; packed [4, 128(p over I), 8(ib in chunk), H] with
        # i = (g*8 + ib)*128 + p, so each chunk is one contiguous 2MB DMA
        dT = np.asarray(down_w[e], dtype=np.float32).T.astype(ml_dtypes.bfloat16)
        dt = np.ascontiguousarray(
            dT.reshape(4, IB // 4, 128, H).transpose(0, 2, 1, 3)
        )
        in_maps.append({"xt": xt, "gut": gut, "dt": dt})

    nc = build_program(C)

    def combine(results):
        out = np.zeros((T, H), dtype=np.float32)
        for e in range(E):
            ix = idxs[e]
            y = results[e]["y"]
            out[ix] += weights[e] * y[: len(ix)]
        return out.reshape(B, S, H)

    return nc, in_maps, combine


def kernel(x, router_w, gate_w, up_w, down_w):
    nc, in_maps, combine = prepare(x, router_w, gate_w, up_w, down_w)
    res = run_bass_kernel_spmd(nc, in_maps, list(range(8)))
    return combine(res.results)


# revision 19
# speedup vs baseline: 1.0508x; 1.0005x over previous
"""Expert-parallel MoE kernel for Trainium2 (8 NeuronCores).

Strategy (matches the expert-parallel sharding hint):
  - Router is evaluated on host with the exact same jax ops as the
    reference (same backend) so top-k decisions match bit-for-bit.
  - Tokens are dispatched (gathered) per expert on host; each of the 8
    cores owns one expert's weights and runs a fused MLP
        Y = (silu(X @ G^T) * (X @ U^T)) @ D^T
    over its gathered tokens in bf16 (fp32 PSUM accumulate).
  - Outputs are combined on host: out[token] += mean_w[e] * Y_e[row].

v2 schedule: tokens are processed in GROUPS of ~1024-1152 (sub-tiles of
<=512 for PSUM) so each gate/up weight block streams from HBM only once
per group (2 passes total at C=2176 instead of 5).  This keeps the
required stage-1 weight bandwidth at ~76 GB/s (vs 153+ before, and 410
for the old 128-token tail, which was DMA-bound).  The down-projection
weights stay SBUF-resident; their (re)load is interleaved into the
middle of the first stage-1 pass so they never delay the critical path.
"""

import sys
from contextlib import ExitStack

if "/opt/trn_rl_repo" not in sys.path:
    sys.path.insert(0, "/opt/trn_rl_repo")

import ml_dtypes
import numpy as np

import concourse.bacc as bacc
import concourse.mybir as mybir
import concourse.tile as tile
from concourse.bass_utils import run_bass_kernel_spmd

B, S, H, I, E, TOPK = 4, 2048, 1024, 4096, 8, 2
T = B * S
KCH = H // 128   # 8 contraction chunks over H
IB = I // 128    # 32 blocks over I
BF16 = mybir.dt.bfloat16
F32 = mybir.dt.float32
ACT_FN = mybir.ActivationFunctionType.Silu

_prog_cache: dict[tuple, object] = {}


def _groups(C):
    """Split capacity C (multiple of 128) into groups of <=1152 tokens,
    each a list of (start, size<=512) sub-tiles."""
    gs = []
    c = 0
    while C - c > 1152:
        gs.append((c, 1024))
        c += 1024
    gs.append((c, C - c))
    out = []
    for g0, gl in gs:
        # near-equal sub-tiles (multiples of 128, <=512) so the matmul free
        # dim stays >=256 and LDWEIGHTS is always hidden (1152 -> 3x384)
        n = -(-gl // 512)
        base = gl // n // 128 * 128
        sizes = [base] * n
        for i in range((gl - base * n) // 128):
            sizes[i] += 128
        subs = []
        s = 0
        for L in sizes:
            subs.append((g0 + s, L))
            s += L
        out.append((g0, gl, subs))
    return out


def build_program(C, reps=1):
    key = (C, reps)
    if key in _prog_cache:
        return _prog_cache[key]
    nc = bacc.Bacc("TRN2", target_bir_lowering=False, debug=False, num_devices=8)

    xt_d = nc.dram_tensor("xt", [128, KCH, C], BF16, kind="ExternalInput").ap()
    gut_d = nc.dram_tensor("gut", [IB, 128, 2, KCH, 128], BF16, kind="ExternalInput").ap()
    dt_d = nc.dram_tensor("dt", [4, 128, IB // 4, H], BF16, kind="ExternalInput").ap()
    y_d = nc.dram_tensor("y", [C, H], F32, kind="ExternalOutput").ap()

    with tile.TileContext(nc) as tc:
        if reps == 1:
            _emit_body(nc, tc, C, xt_d, gut_d, dt_d, y_d)
        else:
            # Manual 2-body unroll: halves the ~2-4us back-edge barriers
            # and lets the second body's lead-in DMAs overlap the first
            # body's stage-2 tail (pool-release deps, no barrier between
            # them).  Each body opens its own pools - the exact structure
            # proven on HW - unlike For_i_unrolled_general (shared pools +
            # rolloff machinery), which faulted the exec unit.
            # hint_engines: body spans many IRAM blocks on PE; the hint
            # makes the back-edge branch I$-hit instead of ~4us.
            pairs, rem = divmod(reps, 2)
            if pairs > 0:
                with tc.For_i(0, pairs, 1, hint_engines=(mybir.EngineType.PE,)):
                    _emit_body(nc, tc, C, xt_d, gut_d, dt_d, y_d)
                    _emit_body(nc, tc, C, xt_d, gut_d, dt_d, y_d)
            for _ in range(rem):
                _emit_body(nc, tc, C, xt_d, gut_d, dt_d, y_d)

    nc.compile()
    _prog_cache[key] = nc
    return nc


def _emit_body(nc, tc, C, xt_d, gut_d, dt_d, y_d):
    groups = _groups(C)
    with (
        tc.tile_pool(name="wpool", bufs=6) as wpool,
        tc.tile_pool(name="xpool", bufs=1) as xpool,
        tc.tile_pool(name="dpool", bufs=1) as dpool,
        tc.tile_pool(name="hpool", bufs=1) as hpool,
        tc.tile_pool(name="spool", bufs=3) as spool,
        tc.tile_pool(name="ypool", bufs=2) as ypool,
        tc.tile_pool(name="psum", bufs=2, space="PSUM") as psum,
    ):
        # D^T resident: 4 chunks x [128, 8, 1024] bf16 (2MB DMAs), loaded
        # lazily: chunk g's DMA is issued mid-way through the first
        # stage-1 pass so it never queues ahead of xt/gut on the wires.
        dts = [
            dpool.tile([128, IB // 4, H], BF16, tag=f"dt{g}", name=f"dt{g}")
            for g in range(4)
        ]
        dt_loaded = [False] * 4

        first = True
        for g0, gl, subs in groups:
            # token activations for this group: one DMA piece per sub-tile so
            # the first sub's matmuls can start before the whole group lands
            # (the first gut block is issued ahead of the bulk of xt).
            xt = xpool.tile([128, KCH, gl], BF16, tag="xt")
            guts = [wpool.tile([128, 2, KCH, 128], BF16, tag="gut", name=f"gut{ib}")
                    for ib in range(2)]
            nc.sync.dma_start(guts[0][:], gut_d[0])
            for s0, sl_len in subs:
                ls = s0 - g0
                nc.sync.dma_start(
                    xt[:, :, ls:ls + sl_len], xt_d[:, :, s0:s0 + sl_len]
                )
            nc.sync.dma_start(guts[1][:], gut_d[1])

            # stage 1: hh[ib][:, c] = silu(G x) * (U x) for the whole group
            hhs = []
            for ib in range(IB):
                if ib < 2:
                    gut = guts[ib]
                else:
                    gut = wpool.tile([128, 2, KCH, 128], BF16, tag="gut")
                    nc.sync.dma_start(gut[:], gut_d[ib])
                if first and ib % 8 == 4:
                    g = ib // 8
                    nc.sync.dma_start(dts[g][:], dt_d[g])
                    dt_loaded[g] = True
                hh = hpool.tile([128, gl], BF16, tag=f"hh{ib}")
                for s0, sl_len in subs:
                    ls = s0 - g0
                    a1 = psum.tile([128, sl_len], F32, tag="a1")
                    for k in range(KCH):
                        nc.tensor.matmul(
                            a1[:], gut[:, 0, k, :], xt[:, k, ls:ls + sl_len],
                            start=(k == 0), stop=(k == KCH - 1),
                        )
                    a2 = psum.tile([128, sl_len], F32, tag="a2")
                    for k in range(KCH):
                        nc.tensor.matmul(
                            a2[:], gut[:, 1, k, :], xt[:, k, ls:ls + sl_len],
                            start=(k == 0), stop=(k == KCH - 1),
                        )
                    sl = spool.tile([128, sl_len], F32, tag="silu")
                    nc.scalar.activation(sl[:], a1[:], ACT_FN)
                    nc.vector.tensor_mul(hh[:, ls:ls + sl_len], sl[:], a2[:])
                hhs.append(hh)
            if first:
                for g in range(4):
                    if not dt_loaded[g]:
                        nc.sync.dma_start(dts[g][:], dt_d[g])
                        dt_loaded[g] = True
                first = False

            # stage 2: Y[c, h] = Hh @ D^T  (contract I), write [w,1024] rows
            for cs0 in range(0, gl, 128):
                w = min(128, gl - cs0)
                yt = ypool.tile([w, H], F32, tag="yt")
                for h0 in range(0, H, 512):
                    py = psum.tile([w, 512], F32, tag="py", bufs=3)
                    for ic in range(IB):
                        nc.tensor.matmul(
                            py[:],
                            hhs[ic][:, cs0:cs0 + w],
                            dts[ic // 8][:, ic % 8, h0:h0 + 512],
                            start=(ic == 0), stop=(ic == IB - 1),
                        )
                    nc.scalar.copy(yt[:, h0:h0 + 512], py[:])
                nc.sync.dma_start(y_d[g0 + cs0:g0 + cs0 + w, :], yt[:])


def _routing(x, router_w):
    """Replicate the reference's routing decisions with identical jax ops."""
    import jax
    import jax.numpy as jnp

    xf = jnp.asarray(x).reshape(-1, H)
    logits = xf @ jnp.asarray(router_w).T
    probs = jax.nn.softmax(logits, axis=-1)
    topk_p, topk_i = jax.lax.top_k(probs, TOPK)
    topk_p = topk_p / topk_p.sum(axis=-1, keepdims=True)
    return np.asarray(topk_p), np.asarray(topk_i)


def prepare(x, router_w, gate_w, up_w, down_w):
    """Host-side dispatch: returns (nc, in_maps, combine) where combine maps
    the per-core device outputs to the full [B,S,H] result."""
    topk_p, topk_i = _routing(x, router_w)
    xf = np.ascontiguousarray(np.asarray(x, dtype=np.float32).reshape(T, H))

    idxs, weights = [], []
    for e in range(E):
        sel = topk_i == e
        mask = sel.any(axis=-1)
        w_tok = (topk_p * sel).sum(axis=-1)
        cnt = int(mask.sum())
        mean_w = float(w_tok.sum() / max(cnt, 1)) if cnt > 0 else 0.0
        idxs.append(np.nonzero(mask)[0])
        weights.append(np.float32(mean_w))

    cmax = max(len(ix) for ix in idxs)
    C = ((cmax + 127) // 128) * 128

    xf_bf = xf.astype(ml_dtypes.bfloat16)
    in_maps = []
    for e in range(E):
        ix = idxs[e]
        # X^T packed [128(p), KCH(k), C(c)] with h = k*128+p
        xt = np.zeros((128, KCH, C), dtype=ml_dtypes.bfloat16)
        xt[:, :, : len(ix)] = xf_bf[ix].T.reshape(KCH, 128, len(ix)).transpose(1, 0, 2)
        # G^T/U^T are [H, I]; packed together as
        # [IB, 128(p), 2(g/u), KCH(k), 128(i)] with h = k*128+p, so each
        # ib block is one contiguous 512KB DMA with 4KB per partition.
        gT = np.asarray(gate_w[e], dtype=np.float32).T.astype(ml_dtypes.bfloat16)
        uT = np.asarray(up_w[e], dtype=np.float32).T.astype(ml_dtypes.bfloat16)
        gt = gT.reshape(KCH, 128, IB, 128).transpose(2, 1, 0, 3)
        ut = uT.reshape(KCH, 128, IB, 128).transpose(2, 1, 0, 3)
        gut = np.ascontiguousarray(np.stack([gt, ut], axis=2))
        # D^T is [I, H]; packed [4, 128(p over I), 8(ib in chunk), H] with
        # i = (g*8 + ib)*128 + p, so each chunk is one contiguous 2MB DMA
        dT = np.asarray(down_w[e], dtype=np.float32).T.astype(ml_dtypes.bfloat16)
        dt = np.ascontiguousarray(
            dT.reshape(4, IB // 4, 128, H).transpose(0, 2, 1, 3)
        )
        in_maps.append({"xt": xt, "gut": gut, "dt": dt})

    nc = build_program(C)

    def combine(results):
        out = np.zeros((T, H), dtype=np.float32)
        for e in range(E):
            ix = idxs[e]
            y = results[e]["y"]
            out[ix] += weights[e] * y[: len(ix)]
        return out.reshape(B, S, H)

    return nc, in_maps, combine


def kernel(x, router_w, gate_w, up_w, down_w):
    nc, in_maps, combine = prepare(x, router_w, gate_w, up_w, down_w)
    res = run_bass_kernel_spmd(nc, in_maps, list(range(8)))
    return combine(res.results)
